# revision 32
# baseline (speedup 1.0000x reference)
"""MoE gate (router) kernel for Trainium2.

Computes, for hidden_states [T, H] and gate weight [E, H]:
    logits = hidden_states @ weight.T          # [T, E]
    probs  = softmax(logits, axis=-1)
    topk_weight, topk_idx = top_k(probs, 8)    # normalized over the top-8
    row_idx = arange(T*8).reshape(8, T).T

Strategy (8 NeuronCores, data parallel over tokens):
  - fp32 accuracy from fp16 hi/lo splits (host-side, same DMA bytes as f32):
    hs = hi + lo/2^11, 64*w = whi + wlo/2^11, with each part fp16 (11-bit
    mantissa, ~22 mantissa bits total; the dropped lo*lo term is ~2^-22).
    fp16 matmul runs 1 cycle/row on the PE (fp32 is 4, fp32r is only ~16
    effective bits on TRN2 silicon -- measured, too coarse for exact top-k).
  - Host packs hs into the exact SBUF tile layout [p][ts][hi/lo][t][ko]
    with the contraction split k = p*56 + ko, so each 128-token tile loads
    with TWO DMAs (hi, lo) of one fully-contiguous 14,336-B descriptor per
    partition (the dominant cost in the naive layout was 229K strided
    256-B descriptors per iteration: sub-512B transfers run at half bus
    rate and per-descriptor overhead swamps the DMA engines).  The p-major
    k split also makes the w pack a pure reshape and the hs pack a
    cache-friendly strided memcpy (112-B runs) on the host.
  - Pipeline fill: a tiny 2-k-tile first weight chunk unblocks the first
    matmul ~6 us in; the rest of the weight load streams interleaved with
    the first token tiles.  Output staging DMAs out in two halves so the
    drain tail overlaps the last tiles' compute.
  - Per k-tile only TWO matmuls: rhs = [whi | wlo] concatenated [128 x 512]
    shares one weight load for the hi*hi and hi*lo terms; the lo*hi term
    accumulates into the same scaled-2^11 PSUM columns as hi*lo:
        psum[:, 0:256]   += hshi . whi
        psum[:, 256:512] += hshi . wlo + hslo . whi
    logits64 = psum[:, 0:256] + 2^-11 * psum[:, 256:512]   (= 64*logits)
  - DVE max/max_index give the top-8 values + indices per token in one
    instruction each.  Softmax over the full 256 experts followed by top-k
    renormalization reduces algebraically to a softmax over just the top-8
    logits; the 1/64 weight pre-scale folds into the exp's scale operand, so
    the full-row softmax and the descale pass are never materialized.
"""

import numpy as np

TOP_K = 8
NUM_EXPERTS = 256
HIDDEN = 7168
NUM_TOKENS = 16384
N_CORES = 8
T_LOC = NUM_TOKENS // N_CORES

W_SCALE = 64.0       # weight pre-scale so fp16(64*w) stays normal-range
LO_SCALE = 2048.0    # 2^11: lo parts carry the next 11 mantissa bits

_NC_CACHE = {}


def build_gate_nc(t_loc=T_LOC, h=HIDDEN, e=NUM_EXPERTS, repeat=1, serialize=False):
    import concourse.mybir as mybir
    import concourse.tile as tile
    from concourse import bacc
    from concourse.tile import add_dep_helper

    f32 = mybir.dt.float32
    fp16 = mybir.dt.float16
    P = 128
    KT = h // P          # k-tiles along hidden dim
    TS = t_loc // P      # 128-token subtiles per core
    # uneven weight DMA chunks: a tiny first chunk unblocks the first
    # matmul as early as possible, the rest stream in behind the first
    # token tiles
    W_CHUNK_LENS = [2, 6, 8, 8, 8, 8, 8, 8]
    assert sum(W_CHUNK_LENS) == KT
    W_CHUNK_STARTS = [sum(W_CHUNK_LENS[:i]) for i in range(len(W_CHUNK_LENS))]

    def k_to_chunk(k):
        for ci in range(len(W_CHUNK_LENS) - 1, -1, -1):
            if k >= W_CHUNK_STARTS[ci]:
                return ci, k - W_CHUNK_STARTS[ci]
        raise AssertionError

    nc = bacc.Bacc("TRN2", target_bir_lowering=False)
    # hs packed on host to the SBUF tile layout: [p][ts][hi/lo][t][ko],
    # with the contraction index split k = p*KT + ko (p-major, so the
    # host-side pack is a cache-friendly copy and w_pack a pure reshape)
    hs_pack = nc.dram_tensor("hs_pack", [P, TS, 2, P, KT], fp16, kind="ExternalInput")
    # w packed to [p][ko][whi|wlo]: w_pack[p, ko, 0:e]=fp16(64*wT), [e:2e]=lo
    w_pack = nc.dram_tensor("w_pack", [P, KT, 2 * e], fp16, kind="ExternalInput")
    idx_out = nc.dram_tensor(
        "topk_idx", [t_loc, TOP_K], mybir.dt.int32, kind="ExternalOutput"
    )
    w_out = nc.dram_tensor("topk_w", [t_loc, TOP_K], f32, kind="ExternalOutput")

    with tile.TileContext(nc) as tc:
        with (
            tc.tile_pool(name="wpool", bufs=1) as wpool,
            tc.tile_pool(name="hpool", bufs=8) as hpool,
            tc.tile_pool(name="lpool", bufs=3) as lpool,
            tc.tile_pool(name="spool", bufs=4) as spool,
            tc.tile_pool(name="psum", bufs=4, space="PSUM") as psum_pool,
        ):
            # output staging: small per-tile results accumulate here and leave
            # as two large descriptor DMAs at the end (tiny per-tile DMAs get
            # the DIRECT2D encoding whose single wait slot walrus overflows)
            if repeat > 0:
                stage_idx = wpool.tile([P, TS, TOP_K], mybir.dt.int32, tag="sidx")
                stage_wv = wpool.tile([P, TS, TOP_K], f32, tag="swv")
            # gate weight: resident in SBUF, chunked so the first matmuls
            # only wait on a tiny first slice of the weight load.  DMA
            # issue order matters: the shared DMA engines drain in
            # dispatch order, so the weight chunks interleave with the
            # first token tiles to grow available PE work fastest.
            wt_chunks = []
            for wc_i, clen in enumerate(W_CHUNK_LENS):
                wc = wpool.tile([P, clen, 2 * e], fp16, tag=f"wt{wc_i}", name=f"wt{wc_i}")
                wt_chunks.append(wc)

            def issue_w(wc_i):
                s0, l0 = W_CHUNK_STARTS[wc_i], W_CHUNK_LENS[wc_i]
                nc.sync.dma_start(wt_chunks[wc_i], w_pack[:, s0 : s0 + l0, :])

            def issue_hs(rep, ts_i):
                # two DMAs per 128-token tile (hi, lo), each one fully
                # contiguous 14,336-B descriptor per partition; the hi
                # half carries 2/3 of the matmul rows so MM1 starts
                # without waiting on the lo half
                hhi = hpool.tile(
                    [P, P, KT], fp16, tag="hs", name=f"hshi{rep}_{ts_i}"
                )
                d = nc.sync.dma_start(hhi, hs_pack[:, ts_i, 0, :, :])
                hlo = hpool.tile(
                    [P, P, KT], fp16, tag="hs", name=f"hslo{rep}_{ts_i}"
                )
                d2 = nc.sync.dma_start(hlo, hs_pack[:, ts_i, 1, :, :])
                return hhi, hlo, d, d2

            # Fill prologue: interleave the weight-chunk DMAs with the
            # first token tiles so PE work (arrived ts x arrived chunks)
            # grows fastest.  ALL weight DMAs are emitted here, before any
            # matmul, so every matmul's read has its producing DMA earlier
            # in program order (the Tile dep tracker only orders
            # read-after-write for earlier writes; a read emitted before
            # its write silently consumes stale SBUF on a cold device).
            N_PRE = min(3, TS) if repeat > 0 else 0
            pre_hs = {}
            dma_done = {}  # analytic completion times under the serialized
            #              # DMA-engine model (bytes / 360 B/ns + dispatch)
            _clock = [0.0]

            def _track(kind, key, nbytes):
                _clock[0] += 0.8e3 * 0 + nbytes / 360.0  # ns
                dma_done[(kind, key)] = _clock[0] + 1500.0  # DGE+sem latency

            HS_BYTES = P * P * KT * 2

            def w_bytes(ci):
                return P * W_CHUNK_LENS[ci] * 2 * e * 2

            # prologue emission = intended DMA drain order
            issue_w(0); _track("w", 0, w_bytes(0))
            if N_PRE > 0:
                pre_hs[0] = issue_hs(0, 0)
                _track("hi", 0, HS_BYTES); _track("lo", 0, HS_BYTES)
            issue_w(1); _track("w", 1, w_bytes(1))
            if N_PRE > 1:
                pre_hs[1] = issue_hs(0, 1)
                _track("hi", 1, HS_BYTES); _track("lo", 1, HS_BYTES)
            issue_w(2); _track("w", 2, w_bytes(2))
            issue_w(3); _track("w", 3, w_bytes(3))
            if N_PRE > 2:
                pre_hs[2] = issue_hs(0, 2)
                _track("hi", 2, HS_BYTES); _track("lo", 2, HS_BYTES)
            issue_w(4); _track("w", 4, w_bytes(4))
            issue_w(5); _track("w", 5, w_bytes(5))
            if N_PRE > 3:
                pre_hs[3] = issue_hs(0, 3)
                _track("hi", 3, HS_BYTES); _track("lo", 3, HS_BYTES)
            issue_w(6); _track("w", 6, w_bytes(6))
            issue_w(7); _track("w", 7, w_bytes(7))

            def mm1(pt, hhi, k, start):
                ci, ki = k_to_chunk(k)
                # psum[:, 0:2e] += hshi . [whi | wlo]
                nc.tensor.matmul(
                    pt, hhi[:, :, k], wt_chunks[ci][:, ki, :], start=start, stop=False
                )

            def mm2(pt, hlo, k, stop):
                ci, ki = k_to_chunk(k)
                # psum[:, e:2e] += hslo . whi  (same 2^11 scale as hi*lo)
                nc.tensor.matmul(
                    pt[:, e:], hlo[:, :, k], wt_chunks[ci][:, ki, :e],
                    start=False, stop=stop,
                )

            # Fill-phase matmuls for the prefetched tiles, emitted in
            # data-ready order: the PE queue is strictly in-order, so ts0's
            # late-chunk matmuls must not sit ahead of ts1's already-ready
            # early-chunk ones.  Each psum's accumulation starts with its
            # ts's first-emitted matmul and stops with its last.
            pre_pt = {}
            if N_PRE:
                sched = []
                for ts_i in range(N_PRE):
                    pre_pt[ts_i] = psum_pool.tile(
                        [P, 2 * e], f32, tag="pt", name=f"pt_pre{ts_i}"
                    )
                    for k in range(KT):
                        ci, _ = k_to_chunk(k)
                        rdy1 = max(dma_done[("w", ci)], dma_done[("hi", ts_i)])
                        rdy2 = max(dma_done[("w", ci)], dma_done[("lo", ts_i)])
                        sched.append((rdy1, ts_i, k, 1))
                        sched.append((rdy2, ts_i, k, 2))
                sched.sort(key=lambda x: (x[0], x[1], x[3], x[2]))
                started = set()
                last_of = {}
                for i, (_, ts_i, k, kind) in enumerate(sched):
                    last_of[ts_i] = i
                for i, (_, ts_i, k, kind) in enumerate(sched):
                    hhi, hlo, _, _ = pre_hs[ts_i]
                    if kind == 1:
                        mm1(pre_pt[ts_i], hhi, k, start=(ts_i not in started))
                        started.add(ts_i)
                        assert i != last_of[ts_i] or KT == 0
                    else:
                        mm2(pre_pt[ts_i], hlo, k, stop=(i == last_of[ts_i]))

            prev_tail = None
            for rep in range(repeat):
                for ts_i in range(TS):
                    if rep == 0 and ts_i < N_PRE:
                        hhi, hlo, d, d2 = pre_hs[ts_i]
                        pt = pre_pt[ts_i]
                    else:
                        hhi, hlo, d, d2 = issue_hs(rep, ts_i)
                        if serialize and prev_tail is not None:
                            add_dep_helper(prev_tail.ins, d.ins, reason="ser-rep")
                            add_dep_helper(prev_tail.ins, d2.ins, reason="ser-rep")
                        pt = psum_pool.tile([P, 2 * e], f32, tag="pt")
                        for k in range(KT):
                            mm1(pt, hhi, k, start=(k == 0))
                            mm2(pt, hlo, k, stop=(k == KT - 1))
                    # logits64 = psum_hi + 2^-11 * psum_cross  (= 64 * logits)
                    logits = lpool.tile([P, e], f32, tag="logits")
                    nc.vector.tensor_scalar_mul(logits, pt[:, e:], 1.0 / LO_SCALE)
                    nc.vector.tensor_add(logits, logits, pt[:, :e])
                    mx = spool.tile([P, TOP_K], f32, tag="mx")
                    nc.vector.max(out=mx, in_=logits)
                    idx_u = spool.tile([P, TOP_K], mybir.dt.uint32, tag="idxu")
                    nc.vector.max_index(idx_u, mx, logits)
                    nc.vector.tensor_copy(stage_idx[:, ts_i, :], idx_u)
                    # normalized top-k softmax on 64x-scaled logits:
                    # exp((v - vmax)/64) / sum
                    nm = spool.tile([P, 1], f32, tag="nm")
                    nc.vector.tensor_scalar_mul(nm, mx[:, 0:1], -1.0 / W_SCALE)
                    ev = spool.tile([P, TOP_K], f32, tag="ev")
                    sm = spool.tile([P, 1], f32, tag="sm")
                    nc.scalar.activation(
                        ev,
                        mx,
                        mybir.ActivationFunctionType.Exp,
                        bias=nm,
                        scale=1.0 / W_SCALE,
                        accum_out=sm,
                    )
                    rc = spool.tile([P, 1], f32, tag="rc")
                    nc.vector.reciprocal(rc, sm)
                    tail = nc.vector.tensor_scalar_mul(stage_wv[:, ts_i, :], ev, rc)
                    if serialize and ts_i == TS - 1:
                        prev_tail = tail
                    # completed output quarters leave while later tiles
                    # still compute, shortening the drain tail
                    if rep == repeat - 1 and (ts_i + 1) % (TS // 4) == 0 and ts_i != TS - 1:
                        q0 = (ts_i + 1 - TS // 4) * P
                        q1 = (ts_i + 1) * P
                        nc.sync.dma_start(
                            idx_out[q0:q1, :].rearrange("(ts p) k -> p ts k", p=P),
                            stage_idx[:, ts_i + 1 - TS // 4 : ts_i + 1, :],
                        )
                        nc.sync.dma_start(
                            w_out[q0:q1, :].rearrange("(ts p) k -> p ts k", p=P),
                            stage_wv[:, ts_i + 1 - TS // 4 : ts_i + 1, :],
                        )
            if repeat > 0:
                q0 = (TS - TS // 4) * P
                nc.sync.dma_start(
                    idx_out[q0:, :].rearrange("(ts p) k -> p ts k", p=P),
                    stage_idx[:, TS - TS // 4 :, :],
                )
                nc.sync.dma_start(
                    w_out[q0:, :].rearrange("(ts p) k -> p ts k", p=P),
                    stage_wv[:, TS - TS // 4 :, :],
                )
    nc.compile()
    return nc


def _get_nc():
    key = (T_LOC, HIDDEN, NUM_EXPERTS)
    if key not in _NC_CACHE:
        _NC_CACHE[key] = build_gate_nc(*key)
    return _NC_CACHE[key]


def _split_fp16(x, pre_scale=1.0):
    """x (f32) -> (hi, lo) fp16 with hi + lo/2^11 ~= pre_scale*x."""
    xs = x * np.float32(pre_scale) if pre_scale != 1.0 else x
    hi = xs.astype(np.float16)
    lo = ((xs - hi.astype(np.float32)) * np.float32(LO_SCALE)).astype(np.float16)
    return hi, lo


def _prep_inputs(hs, w):
    P = 128
    KT = HIDDEN // P
    TS = T_LOC // P
    wT = np.ascontiguousarray(w.T)  # [H, E]
    w_hi, w_lo = _split_fp16(wT, W_SCALE)
    wT_cat = np.concatenate([w_hi, w_lo], axis=1)  # [H, 2E]
    # [H, 2E] -> [p, ko, 2E] with k = p*KT + ko: a pure reshape
    w_pack = wT_cat.reshape(P, KT, 2 * NUM_EXPERTS)
    def pack_core(c):
        hs_c = hs[c * T_LOC : (c + 1) * T_LOC]  # [T_LOC, H]
        hs_hi, hs_lo = _split_fp16(hs_c)        # [T_LOC, H] fp16 each
        # [T_LOC, H] -> [ts, t, p, ko] -> [p, ts, hl, t, ko]; innermost ko
        # runs are 112 contiguous bytes in the source, so the permutation
        # is a cache-friendly strided memcpy rather than a 2-byte gather
        pack = np.empty((P, TS, 2, P, KT), dtype=np.float16)
        for hl, part in enumerate((hs_hi, hs_lo)):
            pack[:, :, hl, :, :] = (
                part.reshape(TS, P, P, KT).transpose(2, 0, 1, 3)
            )
        return {"hs_pack": pack, "w_pack": w_pack}

    from concurrent.futures import ThreadPoolExecutor

    with ThreadPoolExecutor(max_workers=N_CORES) as ex:
        in_maps = list(ex.map(pack_core, range(N_CORES)))
    return in_maps


_FN_CACHE = {}


def _make_runner(nc):
    """Compile a reusable 8-core PJRT callable (same lowering path as
    run_bass_kernel_spmd under axon, but cached so repeat kernel() calls
    skip re-tracing/compiling)."""
    import jax
    import concourse.mybir as mybir
    from concourse import bass2jax
    from jax.sharding import Mesh, NamedSharding, PartitionSpec
    from jax.experimental.shard_map import shard_map

    bass2jax.install_neuronx_cc_hook()
    partition_name = nc.partition_id_tensor.name if nc.partition_id_tensor else None
    in_names, out_names, out_avals, zero_shapes = [], [], [], []
    for alloc in nc.m.functions[0].allocations:
        if not isinstance(alloc, mybir.MemoryLocationSet):
            continue
        name = alloc.memorylocations[0].name
        if alloc.kind == "ExternalInput":
            if name != partition_name:
                in_names.append(name)
        elif alloc.kind == "ExternalOutput":
            shape = tuple(alloc.tensor_shape)
            dtype = mybir.dt.np(alloc.dtype)
            out_names.append(name)
            out_avals.append(jax.core.ShapedArray(shape, dtype))
            zero_shapes.append((shape, dtype))
    n_params = len(in_names)
    n_outs = len(out_avals)
    all_in_names = list(in_names) + list(out_names)
    if partition_name is not None:
        all_in_names.append(partition_name)

    def _body(*args):
        operands = list(args)
        if partition_name is not None:
            operands.append(bass2jax.partition_id_tensor())
        outs = bass2jax._bass_exec_p.bind(
            *operands,
            out_avals=tuple(out_avals),
            in_names=tuple(all_in_names),
            out_names=tuple(out_names),
            lowering_input_output_aliases=(),
            sim_require_finite=True,
            sim_require_nnan=True,
            nc=nc,
        )
        return tuple(outs)

    devices = jax.devices()[:N_CORES]
    mesh = Mesh(np.asarray(devices), ("core",))
    in_specs = (PartitionSpec("core"),) * (n_params + n_outs)
    out_specs = (PartitionSpec("core"),) * len(out_names)
    donate = tuple(range(n_params, n_params + n_outs))
    fn = jax.jit(
        shard_map(
            _body, mesh=mesh, in_specs=in_specs, out_specs=out_specs, check_rep=False
        ),
        donate_argnums=donate,
        keep_unused=True,
    )
    sharding = NamedSharding(mesh, PartitionSpec("core"))

    def run(in_maps):
        concat_in = [
            np.concatenate(
                [np.asarray(in_maps[c][nm]) for c in range(N_CORES)], axis=0
            )
            for nm in in_names
        ]
        zeros = [
            np.zeros((N_CORES * s[0], *s[1:]), dt) for s, dt in zero_shapes
        ]
        dev_in = [jax.device_put(x, sharding) for x in concat_in]
        out_arrs = fn(*dev_in, *zeros)
        return [
            {
                nm: np.asarray(out_arrs[i]).reshape(
                    N_CORES, *out_avals[i].shape
                )[c]
                for i, nm in enumerate(out_names)
            }
            for c in range(N_CORES)
        ]

    return run


def kernel(hidden_states, weight):
    hs = np.asarray(hidden_states, dtype=np.float32)
    w = np.asarray(weight, dtype=np.float32)
    assert hs.shape == (NUM_TOKENS, HIDDEN), hs.shape
    assert w.shape == (NUM_EXPERTS, HIDDEN), w.shape

    in_maps = _prep_inputs(hs, w)
    nc = _get_nc()
    try:
        if "run" not in _FN_CACHE:
            _FN_CACHE["run"] = _make_runner(nc)
        results = _FN_CACHE["run"](in_maps)
    except Exception:
        # fall back to the stock path if the cached-runner path breaks
        from concourse.bass_utils import run_bass_kernel_spmd

        results = run_bass_kernel_spmd(
            nc, in_maps, core_ids=list(range(N_CORES))
        ).results

    topk_idx = np.concatenate([r["topk_idx"] for r in results], axis=0)
    topk_w = np.concatenate([r["topk_w"] for r in results], axis=0)
    row_idx = (
        np.arange(NUM_TOKENS * TOP_K, dtype=np.int32).reshape(TOP_K, NUM_TOKENS).T
    )
    return (
        topk_idx.astype(np.int32),
        topk_w.astype(np.float32),
        row_idx,
    )


# revision 33
# speedup vs baseline: 1.0211x; 1.0211x over previous
"""MoE gate (router) kernel for Trainium2.

Computes, for hidden_states [T, H] and gate weight [E, H]:
    logits = hidden_states @ weight.T          # [T, E]
    probs  = softmax(logits, axis=-1)
    topk_weight, topk_idx = top_k(probs, 8)    # normalized over the top-8
    row_idx = arange(T*8).reshape(8, T).T

Strategy (8 NeuronCores, data parallel over tokens):
  - fp32 accuracy from fp16 hi/lo splits (host-side, same DMA bytes as f32):
    hs = hi + lo/2^11, 64*w = whi + wlo/2^11, with each part fp16 (11-bit
    mantissa, ~22 mantissa bits total; the dropped lo*lo term is ~2^-22).
    fp16 matmul runs 1 cycle/row on the PE (fp32 is 4, fp32r is only ~16
    effective bits on TRN2 silicon -- measured, too coarse for exact top-k).
  - Host packs hs into the exact SBUF tile layout [p][ts][hi/lo][t][ko]
    with the contraction split k = p*56 + ko, so each 128-token tile loads
    with TWO DMAs (hi, lo) of one fully-contiguous 14,336-B descriptor per
    partition (the dominant cost in the naive layout was 229K strided
    256-B descriptors per iteration: sub-512B transfers run at half bus
    rate and per-descriptor overhead swamps the DMA engines).  The p-major
    k split also makes the w pack a pure reshape and the hs pack a
    cache-friendly strided memcpy (112-B runs) on the host.
  - Pipeline fill: a tiny 2-k-tile first weight chunk unblocks the first
    matmul ~6 us in; the rest of the weight load streams interleaved with
    the first token tiles, ALL emitted before any matmul (program order is
    what the Tile dep tracker orders reads against -- a matmul emitted
    before its weight chunk's dma_start reads stale SBUF on a cold device,
    which warm re-runs mask by leaving last run's weights in place).  The
    fill-phase matmuls for the first 3 token tiles are emitted in
    data-ready order since the PE queue executes strictly in order.
    Output staging DMAs out in quarters so the drain tail overlaps the
    last tiles' compute.
  - Per k-tile only TWO matmuls: rhs = [whi | wlo] concatenated [128 x 512]
    shares one weight load for the hi*hi and hi*lo terms; the lo*hi term
    accumulates into the same scaled-2^11 PSUM columns as hi*lo:
        psum[:, 0:256]   += hshi . whi
        psum[:, 256:512] += hshi . wlo + hslo . whi
    logits64 = psum[:, 0:256] + 2^-11 * psum[:, 256:512]   (= 64*logits)
  - DVE max/max_index give the top-8 values + indices per token in one
    instruction each.  Softmax over the full 256 experts followed by top-k
    renormalization reduces algebraically to a softmax over just the top-8
    logits; the 1/64 weight pre-scale folds into the exp's scale operand, so
    the full-row softmax and the descale pass are never materialized.
"""

import numpy as np

TOP_K = 8
NUM_EXPERTS = 256
HIDDEN = 7168
NUM_TOKENS = 16384
N_CORES = 8
T_LOC = NUM_TOKENS // N_CORES

W_SCALE = 64.0       # weight pre-scale so fp16(64*w) stays normal-range
LO_SCALE = 2048.0    # 2^11: lo parts carry the next 11 mantissa bits

_NC_CACHE = {}


def build_gate_nc(t_loc=T_LOC, h=HIDDEN, e=NUM_EXPERTS, repeat=1, serialize=False):
    import concourse.mybir as mybir
    import concourse.tile as tile
    from concourse import bacc
    from concourse.tile import add_dep_helper

    f32 = mybir.dt.float32
    fp16 = mybir.dt.float16
    P = 128
    KT = h // P          # k-tiles along hidden dim
    TS = t_loc // P      # 128-token subtiles per core
    # uneven weight DMA chunks: a tiny first chunk unblocks the first
    # matmul as early as possible, the rest stream in behind the first
    # token tiles
    W_CHUNK_LENS = [2, 6, 8, 8, 8, 8, 8, 8]
    assert sum(W_CHUNK_LENS) == KT
    W_CHUNK_STARTS = [sum(W_CHUNK_LENS[:i]) for i in range(len(W_CHUNK_LENS))]

    def k_to_chunk(k):
        for ci in range(len(W_CHUNK_LENS) - 1, -1, -1):
            if k >= W_CHUNK_STARTS[ci]:
                return ci, k - W_CHUNK_STARTS[ci]
        raise AssertionError

    nc = bacc.Bacc("TRN2", target_bir_lowering=False)
    # hs packed on host to the SBUF tile layout: [p][ts][hi/lo][t][ko],
    # with the contraction index split k = p*KT + ko (p-major, so the
    # host-side pack is a cache-friendly copy and w_pack a pure reshape)
    hs_pack = nc.dram_tensor("hs_pack", [P, TS, 2, P, KT], fp16, kind="ExternalInput")
    # w packed to [p][ko][whi|wlo]: w_pack[p, ko, 0:e]=fp16(64*wT), [e:2e]=lo
    w_pack = nc.dram_tensor("w_pack", [P, KT, 2 * e], fp16, kind="ExternalInput")
    idx_out = nc.dram_tensor(
        "topk_idx", [t_loc, TOP_K], mybir.dt.int32, kind="ExternalOutput"
    )
    w_out = nc.dram_tensor("topk_w", [t_loc, TOP_K], f32, kind="ExternalOutput")

    with tile.TileContext(nc) as tc:
        with (
            tc.tile_pool(name="wpool", bufs=1) as wpool,
            tc.tile_pool(name="hpool", bufs=8) as hpool,
            tc.tile_pool(name="lpool", bufs=3) as lpool,
            tc.tile_pool(name="spool", bufs=4) as spool,
            tc.tile_pool(name="psum", bufs=4, space="PSUM") as psum_pool,
        ):
            # output staging: small per-tile results accumulate here and leave
            # as two large descriptor DMAs at the end (tiny per-tile DMAs get
            # the DIRECT2D encoding whose single wait slot walrus overflows)
            if repeat > 0:
                stage_idx = wpool.tile([P, TS, TOP_K], mybir.dt.int32, tag="sidx")
                stage_wv = wpool.tile([P, TS, TOP_K], f32, tag="swv")
            # gate weight: resident in SBUF, chunked so the first matmuls
            # only wait on a tiny first slice of the weight load.  DMA
            # issue order matters: the shared DMA engines drain in
            # dispatch order, so the weight chunks interleave with the
            # first token tiles to grow available PE work fastest.
            wt_chunks = []
            for wc_i, clen in enumerate(W_CHUNK_LENS):
                wc = wpool.tile([P, clen, 2 * e], fp16, tag=f"wt{wc_i}", name=f"wt{wc_i}")
                wt_chunks.append(wc)

            def issue_w(wc_i):
                s0, l0 = W_CHUNK_STARTS[wc_i], W_CHUNK_LENS[wc_i]
                nc.sync.dma_start(wt_chunks[wc_i], w_pack[:, s0 : s0 + l0, :])

            def issue_hs(rep, ts_i):
                # two DMAs per 128-token tile (hi, lo), each one fully
                # contiguous 14,336-B descriptor per partition; the hi
                # half carries 2/3 of the matmul rows so MM1 starts
                # without waiting on the lo half
                hhi = hpool.tile(
                    [P, P, KT], fp16, tag="hs", name=f"hshi{rep}_{ts_i}"
                )
                d = nc.sync.dma_start(hhi, hs_pack[:, ts_i, 0, :, :])
                hlo = hpool.tile(
                    [P, P, KT], fp16, tag="hs", name=f"hslo{rep}_{ts_i}"
                )
                d2 = nc.sync.dma_start(hlo, hs_pack[:, ts_i, 1, :, :])
                return hhi, hlo, d, d2

            # Fill prologue: interleave the weight-chunk DMAs with the
            # first token tiles so PE work (arrived ts x arrived chunks)
            # grows fastest.  ALL weight DMAs are emitted here, before any
            # matmul, so every matmul's read has its producing DMA earlier
            # in program order (the Tile dep tracker only orders
            # read-after-write for earlier writes; a read emitted before
            # its write silently consumes stale SBUF on a cold device).
            N_PRE = min(3, TS) if repeat > 0 else 0
            pre_hs = {}
            dma_done = {}  # analytic completion times under the serialized
            #              # DMA-engine model (bytes / 360 B/ns + dispatch)
            _clock = [0.0]

            def _track(kind, key, nbytes):
                _clock[0] += 0.8e3 * 0 + nbytes / 360.0  # ns
                dma_done[(kind, key)] = _clock[0] + 1500.0  # DGE+sem latency

            HS_BYTES = P * P * KT * 2

            def w_bytes(ci):
                return P * W_CHUNK_LENS[ci] * 2 * e * 2

            # prologue emission = intended DMA drain order
            issue_w(0); _track("w", 0, w_bytes(0))
            if N_PRE > 0:
                pre_hs[0] = issue_hs(0, 0)
                _track("hi", 0, HS_BYTES); _track("lo", 0, HS_BYTES)
            issue_w(1); _track("w", 1, w_bytes(1))
            if N_PRE > 1:
                pre_hs[1] = issue_hs(0, 1)
                _track("hi", 1, HS_BYTES); _track("lo", 1, HS_BYTES)
            issue_w(2); _track("w", 2, w_bytes(2))
            issue_w(3); _track("w", 3, w_bytes(3))
            if N_PRE > 2:
                pre_hs[2] = issue_hs(0, 2)
                _track("hi", 2, HS_BYTES); _track("lo", 2, HS_BYTES)
            issue_w(4); _track("w", 4, w_bytes(4))
            issue_w(5); _track("w", 5, w_bytes(5))
            if N_PRE > 3:
                pre_hs[3] = issue_hs(0, 3)
                _track("hi", 3, HS_BYTES); _track("lo", 3, HS_BYTES)
            issue_w(6); _track("w", 6, w_bytes(6))
            issue_w(7); _track("w", 7, w_bytes(7))

            def mm1(pt, hhi, k, start):
                ci, ki = k_to_chunk(k)
                # psum[:, 0:2e] += hshi . [whi | wlo]
                nc.tensor.matmul(
                    pt, hhi[:, :, k], wt_chunks[ci][:, ki, :], start=start, stop=False
                )

            def mm2(pt, hlo, k, stop):
                ci, ki = k_to_chunk(k)
                # psum[:, e:2e] += hslo . whi  (same 2^11 scale as hi*lo)
                nc.tensor.matmul(
                    pt[:, e:], hlo[:, :, k], wt_chunks[ci][:, ki, :e],
                    start=False, stop=stop,
                )

            # Fill-phase matmuls for the prefetched tiles, emitted in
            # data-ready order: the PE queue is strictly in-order, so ts0's
            # late-chunk matmuls must not sit ahead of ts1's already-ready
            # early-chunk ones.  Each psum's accumulation starts with its
            # ts's first-emitted matmul and stops with its last.
            pre_pt = {}
            if N_PRE:
                sched = []
                for ts_i in range(N_PRE):
                    pre_pt[ts_i] = psum_pool.tile(
                        [P, 2 * e], f32, tag="pt", name=f"pt_pre{ts_i}"
                    )
                    for k in range(KT):
                        ci, _ = k_to_chunk(k)
                        rdy1 = max(dma_done[("w", ci)], dma_done[("hi", ts_i)])
                        rdy2 = max(dma_done[("w", ci)], dma_done[("lo", ts_i)])
                        sched.append((rdy1, ts_i, k, 1))
                        sched.append((rdy2, ts_i, k, 2))
                sched.sort(key=lambda x: (x[0], x[1], x[3], x[2]))
                started = set()
                last_of = {}
                for i, (_, ts_i, k, kind) in enumerate(sched):
                    last_of[ts_i] = i
                for i, (_, ts_i, k, kind) in enumerate(sched):
                    hhi, hlo, _, _ = pre_hs[ts_i]
                    if kind == 1:
                        mm1(pre_pt[ts_i], hhi, k, start=(ts_i not in started))
                        started.add(ts_i)
                        assert i != last_of[ts_i] or KT == 0
                    else:
                        mm2(pre_pt[ts_i], hlo, k, stop=(i == last_of[ts_i]))

            prev_tail = None
            for rep in range(repeat):
                for ts_i in range(TS):
                    if rep == 0 and ts_i < N_PRE:
                        hhi, hlo, d, d2 = pre_hs[ts_i]
                        pt = pre_pt[ts_i]
                    else:
                        hhi, hlo, d, d2 = issue_hs(rep, ts_i)
                        if serialize and prev_tail is not None:
                            add_dep_helper(prev_tail.ins, d.ins, reason="ser-rep")
                            add_dep_helper(prev_tail.ins, d2.ins, reason="ser-rep")
                        pt = psum_pool.tile([P, 2 * e], f32, tag="pt")
                        for k in range(KT):
                            mm1(pt, hhi, k, start=(k == 0))
                            mm2(pt, hlo, k, stop=(k == KT - 1))
                    # logits64 = psum_hi + 2^-11 * psum_cross  (= 64 * logits)
                    logits = lpool.tile([P, e], f32, tag="logits")
                    nc.vector.tensor_scalar_mul(logits, pt[:, e:], 1.0 / LO_SCALE)
                    nc.vector.tensor_add(logits, logits, pt[:, :e])
                    mx = spool.tile([P, TOP_K], f32, tag="mx")
                    nc.vector.max(out=mx, in_=logits)
                    idx_u = spool.tile([P, TOP_K], mybir.dt.uint32, tag="idxu")
                    nc.vector.max_index(idx_u, mx, logits)
                    nc.vector.tensor_copy(stage_idx[:, ts_i, :], idx_u)
                    # normalized top-k softmax on 64x-scaled logits:
                    # exp((v - vmax)/64) / sum
                    nm = spool.tile([P, 1], f32, tag="nm")
                    nc.vector.tensor_scalar_mul(nm, mx[:, 0:1], -1.0 / W_SCALE)
                    ev = spool.tile([P, TOP_K], f32, tag="ev")
                    sm = spool.tile([P, 1], f32, tag="sm")
                    nc.scalar.activation(
                        ev,
                        mx,
                        mybir.ActivationFunctionType.Exp,
                        bias=nm,
                        scale=1.0 / W_SCALE,
                        accum_out=sm,
                    )
                    rc = spool.tile([P, 1], f32, tag="rc")
                    nc.vector.reciprocal(rc, sm)
                    tail = nc.vector.tensor_scalar_mul(stage_wv[:, ts_i, :], ev, rc)
                    if serialize and ts_i == TS - 1:
                        prev_tail = tail
                    # completed output quarters leave while later tiles
                    # still compute, shortening the drain tail
                    if rep == repeat - 1 and (ts_i + 1) % (TS // 4) == 0 and ts_i != TS - 1:
                        q0 = (ts_i + 1 - TS // 4) * P
                        q1 = (ts_i + 1) * P
                        nc.sync.dma_start(
                            idx_out[q0:q1, :].rearrange("(ts p) k -> p ts k", p=P),
                            stage_idx[:, ts_i + 1 - TS // 4 : ts_i + 1, :],
                        )
                        nc.sync.dma_start(
                            w_out[q0:q1, :].rearrange("(ts p) k -> p ts k", p=P),
                            stage_wv[:, ts_i + 1 - TS // 4 : ts_i + 1, :],
                        )
            if repeat > 0:
                q0 = (TS - TS // 4) * P
                nc.sync.dma_start(
                    idx_out[q0:, :].rearrange("(ts p) k -> p ts k", p=P),
                    stage_idx[:, TS - TS // 4 :, :],
                )
                nc.sync.dma_start(
                    w_out[q0:, :].rearrange("(ts p) k -> p ts k", p=P),
                    stage_wv[:, TS - TS // 4 :, :],
                )
    nc.compile()
    return nc


def _get_nc():
    key = (T_LOC, HIDDEN, NUM_EXPERTS)
    if key not in _NC_CACHE:
        _NC_CACHE[key] = build_gate_nc(*key)
    return _NC_CACHE[key]


def _split_fp16(x, pre_scale=1.0):
    """x (f32) -> (hi, lo) fp16 with hi + lo/2^11 ~= pre_scale*x."""
    xs = x * np.float32(pre_scale) if pre_scale != 1.0 else x
    hi = xs.astype(np.float16)
    lo = ((xs - hi.astype(np.float32)) * np.float32(LO_SCALE)).astype(np.float16)
    return hi, lo


def _prep_inputs(hs, w):
    P = 128
    KT = HIDDEN // P
    TS = T_LOC // P
    wT = np.ascontiguousarray(w.T)  # [H, E]
    w_hi, w_lo = _split_fp16(wT, W_SCALE)
    wT_cat = np.concatenate([w_hi, w_lo], axis=1)  # [H, 2E]
    # [H, 2E] -> [p, ko, 2E] with k = p*KT + ko: a pure reshape
    w_pack = wT_cat.reshape(P, KT, 2 * NUM_EXPERTS)
    def pack_core(c):
        hs_c = hs[c * T_LOC : (c + 1) * T_LOC]  # [T_LOC, H]
        hs_hi, hs_lo = _split_fp16(hs_c)        # [T_LOC, H] fp16 each
        # [T_LOC, H] -> [ts, t, p, ko] -> [p, ts, hl, t, ko]; innermost ko
        # runs are 112 contiguous bytes in the source, so the permutation
        # is a cache-friendly strided memcpy rather than a 2-byte gather
        pack = np.empty((P, TS, 2, P, KT), dtype=np.float16)
        for hl, part in enumerate((hs_hi, hs_lo)):
            pack[:, :, hl, :, :] = (
                part.reshape(TS, P, P, KT).transpose(2, 0, 1, 3)
            )
        return {"hs_pack": pack, "w_pack": w_pack}

    from concurrent.futures import ThreadPoolExecutor

    with ThreadPoolExecutor(max_workers=N_CORES) as ex:
        in_maps = list(ex.map(pack_core, range(N_CORES)))
    return in_maps


_FN_CACHE = {}


def _make_runner(nc):
    """Compile a reusable 8-core PJRT callable (same lowering path as
    run_bass_kernel_spmd under axon, but cached so repeat kernel() calls
    skip re-tracing/compiling)."""
    import jax
    import concourse.mybir as mybir
    from concourse import bass2jax
    from jax.sharding import Mesh, NamedSharding, PartitionSpec
    from jax.experimental.shard_map import shard_map

    bass2jax.install_neuronx_cc_hook()
    partition_name = nc.partition_id_tensor.name if nc.partition_id_tensor else None
    in_names, out_names, out_avals, zero_shapes = [], [], [], []
    for alloc in nc.m.functions[0].allocations:
        if not isinstance(alloc, mybir.MemoryLocationSet):
            continue
        name = alloc.memorylocations[0].name
        if alloc.kind == "ExternalInput":
            if name != partition_name:
                in_names.append(name)
        elif alloc.kind == "ExternalOutput":
            shape = tuple(alloc.tensor_shape)
            dtype = mybir.dt.np(alloc.dtype)
            out_names.append(name)
            out_avals.append(jax.core.ShapedArray(shape, dtype))
            zero_shapes.append((shape, dtype))
    n_params = len(in_names)
    n_outs = len(out_avals)
    all_in_names = list(in_names) + list(out_names)
    if partition_name is not None:
        all_in_names.append(partition_name)

    def _body(*args):
        operands = list(args)
        if partition_name is not None:
            operands.append(bass2jax.partition_id_tensor())
        outs = bass2jax._bass_exec_p.bind(
            *operands,
            out_avals=tuple(out_avals),
            in_names=tuple(all_in_names),
            out_names=tuple(out_names),
            lowering_input_output_aliases=(),
            sim_require_finite=True,
            sim_require_nnan=True,
            nc=nc,
        )
        return tuple(outs)

    devices = jax.devices()[:N_CORES]
    mesh = Mesh(np.asarray(devices), ("core",))
    in_specs = (PartitionSpec("core"),) * (n_params + n_outs)
    out_specs = (PartitionSpec("core"),) * len(out_names)
    donate = tuple(range(n_params, n_params + n_outs))
    fn = jax.jit(
        shard_map(
            _body, mesh=mesh, in_specs=in_specs, out_specs=out_specs, check_rep=False
        ),
        donate_argnums=donate,
        keep_unused=True,
    )
    sharding = NamedSharding(mesh, PartitionSpec("core"))

    def run(in_maps):
        concat_in = [
            np.concatenate(
                [np.asarray(in_maps[c][nm]) for c in range(N_CORES)], axis=0
            )
            for nm in in_names
        ]
        zeros = [
            np.zeros((N_CORES * s[0], *s[1:]), dt) for s, dt in zero_shapes
        ]
        dev_in = [jax.device_put(x, sharding) for x in concat_in]
        out_arrs = fn(*dev_in, *zeros)
        return [
            {
                nm: np.asarray(out_arrs[i]).reshape(
                    N_CORES, *out_avals[i].shape
                )[c]
                for i, nm in enumerate(out_names)
            }
            for c in range(N_CORES)
        ]

    return run


def kernel(hidden_states, weight):
    hs = np.asarray(hidden_states, dtype=np.float32)
    w = np.asarray(weight, dtype=np.float32)
    assert hs.shape == (NUM_TOKENS, HIDDEN), hs.shape
    assert w.shape == (NUM_EXPERTS, HIDDEN), w.shape

    in_maps = _prep_inputs(hs, w)
    nc = _get_nc()
    try:
        if "run" not in _FN_CACHE:
            _FN_CACHE["run"] = _make_runner(nc)
        results = _FN_CACHE["run"](in_maps)
    except Exception:
        # fall back to the stock path if the cached-runner path breaks
        from concourse.bass_utils import run_bass_kernel_spmd

        results = run_bass_kernel_spmd(
            nc, in_maps, core_ids=list(range(N_CORES))
        ).results

    topk_idx = np.concatenate([r["topk_idx"] for r in results], axis=0)
    topk_w = np.concatenate([r["topk_w"] for r in results], axis=0)
    row_idx = (
        np.arange(NUM_TOKENS * TOP_K, dtype=np.int32).reshape(TOP_K, NUM_TOKENS).T
    )
    return (
        topk_idx.astype(np.int32),
        topk_w.astype(np.float32),
        row_idx,
    )
